# revision 9
# baseline (speedup 1.0000x reference)
"""Trainium2 Bass kernel for nn_CAKernel_47459388621075.

10 steps of x = clip(x + 0.1*relu(conv5x5_circular(x, W)), 0, 1) on
x:(16,3,1024,1024) f32, W:(3,3,5,5) f32.

Sharding: batch-parallel over 8 NeuronCores (2 images/core) — the circular
conv is per-image, so no cross-core communication is needed.

Per-core kernel: the whole state lives in SBUF as fp16 for all 10 steps
(12.6 MB; no HBM round-trip per step). Each image is split into 27 row
blocks (26x38 + 1x36 rows). A block's tile [3B+12, 1028] holds the state
rows r-major channel-interleaved (p = 3r + ci) plus 12 window-halo
partitions (rows B,B+1 then rows -2,-1, same interleave) and 2+2 circular
column halos, so every neighbor-halo refresh is a single contiguous
SBUF->SBUF DMA.

Per block per step: 10 fp16 matmuls (5 kernel-column taps x 2 PSUM groups
of 512 cols, dx-major so consecutive matmuls share the stationary matrix),
ACT relu+scale from PSUM, DVE add + clip-min writing the state in place,
2 DVE column-halo copies, and 2 small halo DMAs feeding the next step.
The banded stationary [3B+12, 5*3B] encodes all (ci, dy) taps; out
m = 3r + co matches the state layout so everything is lane-aligned.

Step 0 reads pre-swizzled fp16 records (host-prepared); the last step
writes f32 to HBM in (img, h, co, w) order, transposed back on host.
"""
import sys

sys.path.insert(0, "/opt/trn_rl_repo")

import numpy as np

N_CORES = 8
H = 1024
WC = 1024
WF = WC + 4
CG = 512
BLKS = [38] * 26 + [36]
NB = len(BLKS)
R0S = [0]
for _b in BLKS:
    R0S.append(R0S[-1] + _b)
assert R0S[-1] == H


def make_lhsT(W: np.ndarray, B: int) -> np.ndarray:
    """lhsT[p, 5*(3B)]: window partition p -> out m = 3r + co, per dx.

    Window layout (r-major, channel-interleaved): state row v in [0,B) at
    p = 3v + ci; next-halo rows v in {B,B+1} at p = 3B + 3(v-B) + ci;
    prev-halo rows v in {-2,-1} at p = 3B+6 + 3(v+2) + ci.
    """
    assert W.shape == (3, 3, 5, 5)
    KP = 3 * B + 12
    MP = 3 * B
    lhsT = np.zeros((KP, 5, MP), dtype=np.float32)
    for r in range(B):
        for dy in range(5):
            v = r + dy - 2
            for ci in range(3):
                if 0 <= v < B:
                    p = 3 * v + ci
                elif v >= B:
                    p = 3 * B + 3 * (v - B) + ci
                else:
                    p = 3 * B + 6 + 3 * (v + 2) + ci
                for dx in range(5):
                    for co in range(3):
                        lhsT[p, dx, 3 * r + co] = W[co, ci, dy, dx]
    return lhsT.reshape(KP, 5 * MP)


def prep_x(x: np.ndarray) -> np.ndarray:
    """(n,3,H,W) f32 -> (n, NB, 126, WF) bf16 block records in tile layout."""
    n = x.shape[0]
    xb = x.astype(np.float16)
    out = np.zeros((n, NB, 126, WF), dtype=np.float16)
    for b, B in enumerate(BLKS):
        r0 = R0S[b]
        rows = [(r0 + v) % H for v in range(B)]
        rows += [(r0 + B) % H, (r0 + B + 1) % H, (r0 - 2) % H, (r0 - 1) % H]
        # (n, 3, len(rows), W) -> r-major interleave (n, 3*len(rows), W)
        rec = xb[:, :, rows, :].transpose(0, 2, 1, 3).reshape(n, 3 * len(rows), WC)
        KP = 3 * B + 12
        out[:, b, :KP, 2 : WC + 2] = rec
        out[:, b, :KP, 0:2] = rec[:, :, WC - 2 : WC]
        out[:, b, :KP, WC + 2 : WC + 4] = rec[:, :, 0:2]
    return out


def build_body(tc, xprep_ap, lw38_ap, lw36_ap, y_ap, n_img, steps):
    from contextlib import ExitStack

    from concourse import mybir

    nc = tc.nc
    f32 = mybir.dt.float32
    f16 = mybir.dt.float16
    Relu = mybir.ActivationFunctionType.Relu

    ctx = ExitStack()
    const_pool = ctx.enter_context(tc.tile_pool(name="const", bufs=1))
    state_pool = ctx.enter_context(tc.tile_pool(name="state", bufs=1))
    t_pool = ctx.enter_context(tc.tile_pool(name="t", bufs=6))
    u_pool = ctx.enter_context(tc.tile_pool(name="u", bufs=6))
    y_pool = ctx.enter_context(tc.tile_pool(name="yst", bufs=6))
    psum_pool = ctx.enter_context(tc.tile_pool(name="psum", bufs=3, space="PSUM"))

    lw38 = const_pool.tile([126, 5 * 114], f16)
    nc.sync.dma_start(lw38[:], lw38_ap[:, :])
    lw36 = const_pool.tile([120, 5 * 108], f16)
    nc.sync.dma_start(lw36[:], lw36_ap[:, :])

    # p-state warmup: junk matmuls gated only on the lw38 load keep the PE
    # busy through its frequency ramp while the state tiles stream in
    warm = psum_pool.tile([114, CG], f32, bufs=1)
    for _ in range(8):
        nc.tensor.matmul(warm[:], lw38[0:126, 0:114], lw38[0:126, 0:CG])

    # persistent per-block state tiles (bufs=1 + unique names -> one slot
    # each), loaded once from the host-prepped records; the first two ride
    # the lower-latency HWDGE path so block 0 can start sooner
    state = [
        [state_pool.tile([126, WF], f16, name=f"st{img}_{b}") for b in range(NB)]
        for img in range(n_img)
    ]
    for img in range(n_img):
        for b in range(NB):
            KP = 3 * BLKS[b] + 12
            eng = nc.sync if img == 0 and b < 2 else nc.gpsimd
            eng.dma_start(state[img][b][0:KP, :], xprep_ap[img, b, 0:KP, :])

    for s in range(steps):
        last = s == steps - 1
        for img in range(n_img):
            for b in range(NB):
                B = BLKS[b]
                KP = 3 * B + 12
                MP = 3 * B
                R0 = R0S[b]
                lw = lw38 if B == 38 else lw36
                st = state[img][b]

                psums = [psum_pool.tile([114, CG], f32, name=f"ps{g}") for g in range(2)]
                for dx in range(5):
                    for g in range(2):
                        nc.tensor.matmul(
                            psums[g][0:MP, :],
                            lw[0:KP, MP * dx : MP * (dx + 1)],
                            st[0:KP, g * CG + dx : g * CG + dx + CG],
                            start=(dx == 0),
                            stop=(dx == 4),
                        )
                t = t_pool.tile([114, WC], f16)
                for g in range(2):
                    nc.scalar.activation(
                        t[0:MP, g * CG : (g + 1) * CG], psums[g][0:MP, :], Relu,
                        scale=0.1,
                    )
                u = u_pool.tile([114, WC], f16)
                nc.vector.tensor_add(u[0:MP, :], t[0:MP, :], st[0:MP, 2 : WC + 2])
                if last:
                    yt = y_pool.tile([114, WC], f32)
                    nc.vector.tensor_scalar_min(yt[0:MP, :], u[0:MP, :], 1.0)
                    nc.gpsimd.dma_start(y_ap[img, R0 : R0 + B, :, :], yt[0:MP, :])
                    continue
                nc.vector.tensor_scalar_min(st[0:MP, 2 : WC + 2], u[0:MP, :], 1.0)
                nc.vector.tensor_copy(st[0:MP, 0:2], st[0:MP, WC : WC + 2])
                nc.vector.tensor_copy(st[0:MP, WC + 2 : WC + 4], st[0:MP, 2:4])
                # halo refreshes feeding step s+1 (single contiguous DMAs
                # thanks to the r-major interleave)
                if b >= 1:
                    Bp = BLKS[b - 1]
                    stp = state[img][b - 1]
                    nc.sync.dma_start(stp[3 * Bp : 3 * Bp + 6, :], st[0:6, :])
                    nc.sync.dma_start(
                        st[3 * B + 6 : 3 * B + 12, :], stp[3 * Bp - 6 : 3 * Bp, :]
                    )
                if b == NB - 1:
                    B0 = BLKS[0]
                    st0 = state[img][0]
                    nc.sync.dma_start(st[3 * B : 3 * B + 6, :], st0[0:6, :])
                    nc.sync.dma_start(
                        st0[3 * B0 + 6 : 3 * B0 + 12, :], st[3 * B - 6 : 3 * B, :]
                    )

    ctx.close()


def _dedup_ldweights(nc):
    """Drop InstLdweights whose weights AP matches the immediately preceding
    load in the same block (the PE array retains the stationary between
    matmuls). The tile legalizer emits one load per matmul; dx-major emission
    makes every second one redundant. Only sync-free loads are removed."""
    removed = 0
    for fn in nc.m.functions:
        for bb in fn.blocks:
            out = []
            last_key = None
            for inst in bb.instructions:
                tn = type(inst).__name__
                if tn == "InstLdweights":
                    ap = inst.ins[0]
                    key = (
                        getattr(ap, "memref", None),
                        ap.offset,
                        str(ap.ap),
                        str(inst.perf_mode),
                        str(inst.is_transpose),
                        str(inst.tile_position),
                        str(inst.tile_size),
                    )
                    si = inst.sync_info
                    clean = si is None or (
                        len(si.on_wait) == 0 and len(si.on_update) == 0
                    )
                    if clean and key == last_key:
                        removed += 1
                        continue
                    last_key = key
                elif tn not in ("InstMatmult", "InstEventSemaphore", "InstBranchHint"):
                    if getattr(inst, "engine", None) is not None and str(
                        inst.engine
                    ).endswith("PE"):
                        last_key = None
                out.append(inst)
            bb.instructions = out
    return removed


_PROGRAM_CACHE = {}


def _build_program(n_img, steps):
    key = (n_img, steps)
    if key in _PROGRAM_CACHE:
        return _PROGRAM_CACHE[key]
    import concourse.tile as tile
    from concourse import bacc, mybir

    nc = bacc.Bacc(
        "TRN2",
        target_bir_lowering=False,
        debug=False,
        enable_asserts=False,
        num_devices=N_CORES,
    )
    f32 = mybir.dt.float32
    f16 = mybir.dt.float16
    xprep_ap = nc.dram_tensor(
        "xprep", (n_img, NB, 126, WF), f16, kind="ExternalInput"
    ).ap()
    lw38_ap = nc.dram_tensor("lw38", (126, 5 * 114), f16, kind="ExternalInput").ap()
    lw36_ap = nc.dram_tensor("lw36", (120, 5 * 108), f16, kind="ExternalInput").ap()
    y_ap = nc.dram_tensor("y", (n_img, H, 3, WC), f32, kind="ExternalOutput").ap()
    with tile.TileContext(nc) as tc:
        build_body(tc, xprep_ap, lw38_ap, lw36_ap, y_ap, n_img, steps)
    nc.compile()
    _dedup_ldweights(nc)
    _PROGRAM_CACHE[key] = nc
    return nc


def kernel(x: np.ndarray, W: np.ndarray, steps) -> np.ndarray:
    from concourse.bass_utils import run_bass_kernel_spmd

    x = np.ascontiguousarray(np.asarray(x), dtype=np.float32)
    W = np.asarray(W, dtype=np.float32)
    steps = int(steps)
    n, c, Hx, Wx = x.shape
    assert c == 3 and Hx == H and Wx == WC and n % N_CORES == 0
    if steps == 0:
        return x
    per = n // N_CORES

    nc = _build_program(per, steps)
    xprep = prep_x(x)
    lw38 = make_lhsT(W, 38).astype(np.float16)
    lw36 = make_lhsT(W, 36).astype(np.float16)
    in_maps = [
        {"xprep": xprep[i * per : (i + 1) * per], "lw38": lw38, "lw36": lw36}
        for i in range(N_CORES)
    ]
    res = run_bass_kernel_spmd(nc, in_maps, core_ids=list(range(N_CORES)))
    y = np.concatenate([res.results[i]["y"] for i in range(N_CORES)], axis=0)
    return np.ascontiguousarray(y.transpose(0, 2, 1, 3)).astype(np.float32)


# revision 13
# speedup vs baseline: 1.0020x; 1.0020x over previous
"""Trainium2 Bass kernel for nn_CAKernel_47459388621075.

10 steps of x = clip(x + 0.1*relu(conv5x5_circular(x, W)), 0, 1) on
x:(16,3,1024,1024) f32, W:(3,3,5,5) f32.

Sharding: batch-parallel over 8 NeuronCores (2 images/core) — the circular
conv is per-image, so no cross-core communication is needed.

Per-core kernel: the whole state lives in SBUF as fp16 for all 10 steps
(12.6 MB; no HBM round-trip per step). Each image is split into 27 row
blocks (26x38 + 1x36 rows). A block's tile [3B+12, 1028] holds the state
rows r-major channel-interleaved (p = 3r + ci) plus 12 window-halo
partitions (rows B,B+1 then rows -2,-1, same interleave) and 2+2 circular
column halos, so every neighbor-halo refresh is a single contiguous
SBUF->SBUF DMA.

Per block per step: 10 fp16 matmuls (5 kernel-column taps x 2 PSUM groups
of 512 cols, dx-major so consecutive matmuls share the stationary matrix),
ACT relu+scale from PSUM, DVE add + clip-min writing the state in place,
2 DVE column-halo copies, and 2 small halo DMAs feeding the next step.
The banded stationary [3B+12, 5*3B] encodes all (ci, dy) taps; out
m = 3r + co matches the state layout so everything is lane-aligned.

Step 0 reads pre-swizzled fp16 records (host-prepared); the last step
writes f32 to HBM in (img, h, co, w) order, transposed back on host.
"""
import sys

sys.path.insert(0, "/opt/trn_rl_repo")

import numpy as np

N_CORES = 8
H = 1024
WC = 1024
WF = WC + 4
CG = 512
BLKS = [38] * 26 + [36]
NB = len(BLKS)
R0S = [0]
for _b in BLKS:
    R0S.append(R0S[-1] + _b)
assert R0S[-1] == H


def make_lhsT(W: np.ndarray, B: int) -> np.ndarray:
    """lhsT[p, 5*(3B)]: window partition p -> out m = 3r + co, per dx.

    Window layout (r-major, channel-interleaved): state row v in [0,B) at
    p = 3v + ci; next-halo rows v in {B,B+1} at p = 3B + 3(v-B) + ci;
    prev-halo rows v in {-2,-1} at p = 3B+6 + 3(v+2) + ci.
    """
    assert W.shape == (3, 3, 5, 5)
    KP = 3 * B + 12
    MP = 3 * B
    lhsT = np.zeros((KP, 5, MP), dtype=np.float32)
    for r in range(B):
        for dy in range(5):
            v = r + dy - 2
            for ci in range(3):
                if 0 <= v < B:
                    p = 3 * v + ci
                elif v >= B:
                    p = 3 * B + 3 * (v - B) + ci
                else:
                    p = 3 * B + 6 + 3 * (v + 2) + ci
                for dx in range(5):
                    for co in range(3):
                        lhsT[p, dx, 3 * r + co] = W[co, ci, dy, dx]
    return lhsT.reshape(KP, 5 * MP)


def prep_x(x: np.ndarray) -> np.ndarray:
    """(n,3,H,W) f32 -> (n, NB, 126, WF) bf16 block records in tile layout."""
    n = x.shape[0]
    xb = x.astype(np.float16)
    out = np.zeros((n, NB, 126, WF), dtype=np.float16)
    for b, B in enumerate(BLKS):
        r0 = R0S[b]
        rows = [(r0 + v) % H for v in range(B)]
        rows += [(r0 + B) % H, (r0 + B + 1) % H, (r0 - 2) % H, (r0 - 1) % H]
        # (n, 3, len(rows), W) -> r-major interleave (n, 3*len(rows), W)
        rec = xb[:, :, rows, :].transpose(0, 2, 1, 3).reshape(n, 3 * len(rows), WC)
        KP = 3 * B + 12
        out[:, b, :KP, 2 : WC + 2] = rec
        out[:, b, :KP, 0:2] = rec[:, :, WC - 2 : WC]
        out[:, b, :KP, WC + 2 : WC + 4] = rec[:, :, 0:2]
    return out


def build_body(tc, xprep_ap, lw38_ap, lw36_ap, y_ap, n_img, steps):
    from contextlib import ExitStack

    from concourse import mybir

    nc = tc.nc
    f32 = mybir.dt.float32
    f16 = mybir.dt.float16
    Relu = mybir.ActivationFunctionType.Relu

    ctx = ExitStack()
    const_pool = ctx.enter_context(tc.tile_pool(name="const", bufs=1))
    state_pool = ctx.enter_context(tc.tile_pool(name="state", bufs=1))
    t_pool = ctx.enter_context(tc.tile_pool(name="t", bufs=6))
    u_pool = ctx.enter_context(tc.tile_pool(name="u", bufs=6))
    y_pool = ctx.enter_context(tc.tile_pool(name="yst", bufs=6))
    psum_pool = ctx.enter_context(tc.tile_pool(name="psum", bufs=4, space="PSUM"))

    lw38 = const_pool.tile([126, 5 * 114], f16)
    nc.sync.dma_start(lw38[:], lw38_ap[:, :])
    lw36 = const_pool.tile([120, 5 * 108], f16)
    nc.sync.dma_start(lw36[:], lw36_ap[:, :])

    # persistent per-block state tiles (bufs=1 + unique names -> one slot
    # each), loaded once from the host-prepped records
    state = [
        [state_pool.tile([126, WF], f16, name=f"st{img}_{b}") for b in range(NB)]
        for img in range(n_img)
    ]
    for img in range(n_img):
        for b in range(NB):
            KP = 3 * BLKS[b] + 12
            nc.gpsimd.dma_start(state[img][b][0:KP, :], xprep_ap[img, b, 0:KP, :])

    for s in range(steps):
        last = s == steps - 1
        for img in range(n_img):
            for b in range(NB):
                B = BLKS[b]
                KP = 3 * B + 12
                MP = 3 * B
                R0 = R0S[b]
                lw = lw38 if B == 38 else lw36
                st = state[img][b]

                psums = [psum_pool.tile([114, CG], f32, name=f"ps{g}") for g in range(2)]
                for dx in range(5):
                    for g in range(2):
                        nc.tensor.matmul(
                            psums[g][0:MP, :],
                            lw[0:KP, MP * dx : MP * (dx + 1)],
                            st[0:KP, g * CG + dx : g * CG + dx + CG],
                            start=(dx == 0),
                            stop=(dx == 4),
                        )
                t = t_pool.tile([114, WC], f16)
                for g in range(2):
                    nc.scalar.activation(
                        t[0:MP, g * CG : (g + 1) * CG], psums[g][0:MP, :], Relu,
                        scale=0.1,
                    )
                u = u_pool.tile([114, WC], f16)
                nc.vector.tensor_add(u[0:MP, :], t[0:MP, :], st[0:MP, 2 : WC + 2])
                if last:
                    yt = y_pool.tile([114, WC], f32)
                    nc.vector.tensor_scalar_min(yt[0:MP, :], u[0:MP, :], 1.0)
                    nc.gpsimd.dma_start(y_ap[img, R0 : R0 + B, :, :], yt[0:MP, :])
                    continue
                nc.vector.tensor_scalar_min(st[0:MP, 2 : WC + 2], u[0:MP, :], 1.0)
                nc.vector.tensor_copy(st[0:MP, 0:2], st[0:MP, WC : WC + 2])
                nc.vector.tensor_copy(st[0:MP, WC + 2 : WC + 4], st[0:MP, 2:4])
                # halo refreshes feeding step s+1 (single contiguous DMAs
                # thanks to the r-major interleave)
                if b >= 1:
                    Bp = BLKS[b - 1]
                    stp = state[img][b - 1]
                    nc.sync.dma_start(stp[3 * Bp : 3 * Bp + 6, :], st[0:6, :])
                    nc.sync.dma_start(
                        st[3 * B + 6 : 3 * B + 12, :], stp[3 * Bp - 6 : 3 * Bp, :]
                    )
                if b == NB - 1:
                    B0 = BLKS[0]
                    st0 = state[img][0]
                    nc.sync.dma_start(st[3 * B : 3 * B + 6, :], st0[0:6, :])
                    nc.sync.dma_start(
                        st0[3 * B0 + 6 : 3 * B0 + 12, :], st[3 * B - 6 : 3 * B, :]
                    )

    ctx.close()


_PROGRAM_CACHE = {}


def _build_program(n_img, steps):
    key = (n_img, steps)
    if key in _PROGRAM_CACHE:
        return _PROGRAM_CACHE[key]
    import concourse.tile as tile
    from concourse import bacc, mybir

    nc = bacc.Bacc(
        "TRN2",
        target_bir_lowering=False,
        debug=False,
        enable_asserts=False,
        num_devices=N_CORES,
    )
    f32 = mybir.dt.float32
    f16 = mybir.dt.float16
    xprep_ap = nc.dram_tensor(
        "xprep", (n_img, NB, 126, WF), f16, kind="ExternalInput"
    ).ap()
    lw38_ap = nc.dram_tensor("lw38", (126, 5 * 114), f16, kind="ExternalInput").ap()
    lw36_ap = nc.dram_tensor("lw36", (120, 5 * 108), f16, kind="ExternalInput").ap()
    y_ap = nc.dram_tensor("y", (n_img, H, 3, WC), f32, kind="ExternalOutput").ap()
    with tile.TileContext(nc) as tc:
        build_body(tc, xprep_ap, lw38_ap, lw36_ap, y_ap, n_img, steps)
    nc.compile()
    _PROGRAM_CACHE[key] = nc
    return nc


def kernel(x: np.ndarray, W: np.ndarray, steps) -> np.ndarray:
    from concourse.bass_utils import run_bass_kernel_spmd

    x = np.ascontiguousarray(np.asarray(x), dtype=np.float32)
    W = np.asarray(W, dtype=np.float32)
    steps = int(steps)
    n, c, Hx, Wx = x.shape
    assert c == 3 and Hx == H and Wx == WC and n % N_CORES == 0
    if steps == 0:
        return x
    per = n // N_CORES

    nc = _build_program(per, steps)
    xprep = prep_x(x)
    lw38 = make_lhsT(W, 38).astype(np.float16)
    lw36 = make_lhsT(W, 36).astype(np.float16)
    in_maps = [
        {"xprep": xprep[i * per : (i + 1) * per], "lw38": lw38, "lw36": lw36}
        for i in range(N_CORES)
    ]
    res = run_bass_kernel_spmd(nc, in_maps, core_ids=list(range(N_CORES)))
    y = np.concatenate([res.results[i]["y"] for i in range(N_CORES)], axis=0)
    return np.ascontiguousarray(y.transpose(0, 2, 1, 3)).astype(np.float32)


# revision 16
# speedup vs baseline: 1.0144x; 1.0124x over previous
"""Trainium2 Bass kernel for nn_CAKernel_47459388621075.

10 steps of x = clip(x + 0.1*relu(conv5x5_circular(x, W)), 0, 1) on
x:(16,3,1024,1024) f32, W:(3,3,5,5) f32.

Sharding: batch-parallel over 8 NeuronCores (2 images/core) — the circular
conv is per-image, so no cross-core communication is needed.

Per-core kernel: the whole state lives in SBUF as fp16 for all 10 steps
(12.6 MB; no HBM round-trip per step). Each image is split into 27 row
blocks (26x38 + 1x36 rows). A block's tile [3B+12, 1028] holds the state
rows r-major channel-interleaved (p = 3r + ci) plus 12 window-halo
partitions (rows B,B+1 then rows -2,-1, same interleave) and 2+2 circular
column halos, so every neighbor-halo refresh is a single contiguous
SBUF->SBUF DMA.

Per block per step: 10 fp16 matmuls (5 kernel-column taps x 2 PSUM groups
of 512 cols, dx-major so consecutive matmuls share the stationary matrix),
ACT relu+scale from PSUM, DVE add + clip-min writing the state in place,
2 DVE column-halo copies, and 2 small halo DMAs feeding the next step.
The banded stationary [3B+12, 5*3B] encodes all (ci, dy) taps; out
m = 3r + co matches the state layout so everything is lane-aligned.

Step 0 reads pre-swizzled fp16 records (host-prepared); the last step
writes f32 to HBM in (img, h, co, w) order, transposed back on host.
"""
import sys

sys.path.insert(0, "/opt/trn_rl_repo")

import numpy as np

N_CORES = 8
H = 1024
WC = 1024
WF = WC + 4
CG = 512
BLKS = [38] * 26 + [36]
NB = len(BLKS)
R0S = [0]
for _b in BLKS:
    R0S.append(R0S[-1] + _b)
assert R0S[-1] == H


def make_lhsT(W: np.ndarray, B: int) -> np.ndarray:
    """lhsT[p, 5*(3B)]: window partition p -> out m = 3r + co, per dx.

    Window layout (r-major, channel-interleaved): state row v in [0,B) at
    p = 3v + ci; next-halo rows v in {B,B+1} at p = 3B + 3(v-B) + ci;
    prev-halo rows v in {-2,-1} at p = 3B+6 + 3(v+2) + ci.
    """
    assert W.shape == (3, 3, 5, 5)
    KP = 3 * B + 12
    MP = 3 * B
    lhsT = np.zeros((KP, 5, MP), dtype=np.float32)
    for r in range(B):
        for dy in range(5):
            v = r + dy - 2
            for ci in range(3):
                if 0 <= v < B:
                    p = 3 * v + ci
                elif v >= B:
                    p = 3 * B + 3 * (v - B) + ci
                else:
                    p = 3 * B + 6 + 3 * (v + 2) + ci
                for dx in range(5):
                    for co in range(3):
                        lhsT[p, dx, 3 * r + co] = W[co, ci, dy, dx]
    return lhsT.reshape(KP, 5 * MP)


def prep_x(x: np.ndarray) -> np.ndarray:
    """(n,3,H,W) f32 -> (n, NB, 126, WF) bf16 block records in tile layout."""
    n = x.shape[0]
    xb = x.astype(np.float16)
    out = np.zeros((n, NB, 126, WF), dtype=np.float16)
    for b, B in enumerate(BLKS):
        r0 = R0S[b]
        rows = [(r0 + v) % H for v in range(B)]
        rows += [(r0 + B) % H, (r0 + B + 1) % H, (r0 - 2) % H, (r0 - 1) % H]
        # (n, 3, len(rows), W) -> r-major interleave (n, 3*len(rows), W)
        rec = xb[:, :, rows, :].transpose(0, 2, 1, 3).reshape(n, 3 * len(rows), WC)
        KP = 3 * B + 12
        out[:, b, :KP, 2 : WC + 2] = rec
        out[:, b, :KP, 0:2] = rec[:, :, WC - 2 : WC]
        out[:, b, :KP, WC + 2 : WC + 4] = rec[:, :, 0:2]
    return out


def build_body(tc, xprep_ap, lw38_ap, lw36_ap, y_ap, n_img, steps):
    from contextlib import ExitStack

    from concourse import mybir

    nc = tc.nc
    f32 = mybir.dt.float32
    f16 = mybir.dt.float16
    Relu = mybir.ActivationFunctionType.Relu

    ctx = ExitStack()
    const_pool = ctx.enter_context(tc.tile_pool(name="const", bufs=1))
    state_pool = ctx.enter_context(tc.tile_pool(name="state", bufs=1))
    t_pool = ctx.enter_context(tc.tile_pool(name="t", bufs=6))
    u_pool = ctx.enter_context(tc.tile_pool(name="u", bufs=6))
    y_pool = ctx.enter_context(tc.tile_pool(name="yst", bufs=6))
    psum_pool = ctx.enter_context(tc.tile_pool(name="psum", bufs=4, space="PSUM"))

    lw38 = const_pool.tile([126, 5 * 114], f16)
    nc.sync.dma_start(lw38[:], lw38_ap[:, :])
    lw36 = const_pool.tile([120, 5 * 108], f16)
    nc.sync.dma_start(lw36[:], lw36_ap[:, :])

    # persistent per-block state tiles (bufs=1 + unique names -> one slot
    # each), loaded once from the host-prepped records
    state = [
        [state_pool.tile([126, WF], f16, name=f"st{img}_{b}") for b in range(NB)]
        for img in range(n_img)
    ]
    for img in range(n_img):
        for b in range(NB):
            KP = 3 * BLKS[b] + 12
            # first loads ride HWDGE: the SWDGE path stalls ~8us on the
            # gpsimd library load at kernel start
            eng = nc.sync if img == 0 and b < 2 else nc.gpsimd
            eng.dma_start(state[img][b][0:KP, :], xprep_ap[img, b, 0:KP, :])

    for s in range(steps):
        last = s == steps - 1
        for img in range(n_img):
            for b in range(NB):
                B = BLKS[b]
                KP = 3 * B + 12
                MP = 3 * B
                R0 = R0S[b]
                lw = lw38 if B == 38 else lw36
                st = state[img][b]

                psums = [psum_pool.tile([114, CG], f32, name=f"ps{g}") for g in range(2)]
                for dx in range(5):
                    for g in range(2):
                        nc.tensor.matmul(
                            psums[g][0:MP, :],
                            lw[0:KP, MP * dx : MP * (dx + 1)],
                            st[0:KP, g * CG + dx : g * CG + dx + CG],
                            start=(dx == 0),
                            stop=(dx == 4),
                        )
                t = t_pool.tile([114, WC], f16)
                for g in range(2):
                    nc.scalar.activation(
                        t[0:MP, g * CG : (g + 1) * CG], psums[g][0:MP, :], Relu,
                        scale=0.1,
                    )
                u = u_pool.tile([114, WC], f16)
                nc.vector.tensor_add(u[0:MP, :], t[0:MP, :], st[0:MP, 2 : WC + 2])
                if last:
                    # fp16 output (host upcasts): keeps the min in the fast
                    # DVE mode and halves store bytes, so the trailing
                    # blocks' store chain doesn't back up after the last
                    # matmul; sync queue is idle in the last step
                    yt = y_pool.tile([114, WC], f16)
                    nc.vector.tensor_scalar_min(yt[0:MP, :], u[0:MP, :], 1.0)
                    nc.sync.dma_start(y_ap[img, R0 : R0 + B, :, :], yt[0:MP, :])
                    continue
                nc.vector.tensor_scalar_min(st[0:MP, 2 : WC + 2], u[0:MP, :], 1.0)
                nc.vector.tensor_copy(st[0:MP, 0:2], st[0:MP, WC : WC + 2])
                nc.vector.tensor_copy(st[0:MP, WC + 2 : WC + 4], st[0:MP, 2:4])
                # halo refreshes feeding step s+1 (single contiguous DMAs
                # thanks to the r-major interleave)
                if b >= 1:
                    Bp = BLKS[b - 1]
                    stp = state[img][b - 1]
                    nc.sync.dma_start(stp[3 * Bp : 3 * Bp + 6, :], st[0:6, :])
                    nc.sync.dma_start(
                        st[3 * B + 6 : 3 * B + 12, :], stp[3 * Bp - 6 : 3 * Bp, :]
                    )
                if b == NB - 1:
                    B0 = BLKS[0]
                    st0 = state[img][0]
                    nc.sync.dma_start(st[3 * B : 3 * B + 6, :], st0[0:6, :])
                    nc.sync.dma_start(
                        st0[3 * B0 + 6 : 3 * B0 + 12, :], st[3 * B - 6 : 3 * B, :]
                    )

    ctx.close()


_PROGRAM_CACHE = {}


def _build_program(n_img, steps):
    key = (n_img, steps)
    if key in _PROGRAM_CACHE:
        return _PROGRAM_CACHE[key]
    import concourse.tile as tile
    from concourse import bacc, mybir

    nc = bacc.Bacc(
        "TRN2",
        target_bir_lowering=False,
        debug=False,
        enable_asserts=False,
        num_devices=N_CORES,
    )
    f32 = mybir.dt.float32
    f16 = mybir.dt.float16
    xprep_ap = nc.dram_tensor(
        "xprep", (n_img, NB, 126, WF), f16, kind="ExternalInput"
    ).ap()
    lw38_ap = nc.dram_tensor("lw38", (126, 5 * 114), f16, kind="ExternalInput").ap()
    lw36_ap = nc.dram_tensor("lw36", (120, 5 * 108), f16, kind="ExternalInput").ap()
    y_ap = nc.dram_tensor("y", (n_img, H, 3, WC), f16, kind="ExternalOutput").ap()
    with tile.TileContext(nc) as tc:
        build_body(tc, xprep_ap, lw38_ap, lw36_ap, y_ap, n_img, steps)
    nc.compile()
    _PROGRAM_CACHE[key] = nc
    return nc


def kernel(x: np.ndarray, W: np.ndarray, steps) -> np.ndarray:
    from concourse.bass_utils import run_bass_kernel_spmd

    x = np.ascontiguousarray(np.asarray(x), dtype=np.float32)
    W = np.asarray(W, dtype=np.float32)
    steps = int(steps)
    n, c, Hx, Wx = x.shape
    assert c == 3 and Hx == H and Wx == WC and n % N_CORES == 0
    if steps == 0:
        return x
    per = n // N_CORES

    nc = _build_program(per, steps)
    xprep = prep_x(x)
    lw38 = make_lhsT(W, 38).astype(np.float16)
    lw36 = make_lhsT(W, 36).astype(np.float16)
    in_maps = [
        {"xprep": xprep[i * per : (i + 1) * per], "lw38": lw38, "lw36": lw36}
        for i in range(N_CORES)
    ]
    res = run_bass_kernel_spmd(nc, in_maps, core_ids=list(range(N_CORES)))
    y = np.concatenate([res.results[i]["y"] for i in range(N_CORES)], axis=0)
    return np.ascontiguousarray(y.transpose(0, 2, 1, 3)).astype(np.float32)


# revision 17
# speedup vs baseline: 1.0192x; 1.0047x over previous
"""Trainium2 Bass kernel for nn_CAKernel_47459388621075.

10 steps of x = clip(x + 0.1*relu(conv5x5_circular(x, W)), 0, 1) on
x:(16,3,1024,1024) f32, W:(3,3,5,5) f32.

Sharding: batch-parallel over 8 NeuronCores (2 images/core) — the circular
conv is per-image, so no cross-core communication is needed.

Per-core kernel: the whole state lives in SBUF as fp16 for all 10 steps
(12.6 MB; no HBM round-trip per step). Each image is split into 27 row
blocks (26x38 + 1x36 rows). A block's tile [3B+12, 1028] holds the state
rows r-major channel-interleaved (p = 3r + ci) plus 12 window-halo
partitions (rows B,B+1 then rows -2,-1, same interleave) and 2+2 circular
column halos, so every neighbor-halo refresh is a single contiguous
SBUF->SBUF DMA.

Per block per step: 10 fp16 matmuls (5 kernel-column taps x 2 PSUM groups
of 512 cols, dx-major so consecutive matmuls share the stationary matrix),
ACT relu+scale from PSUM, DVE add + clip-min writing the state in place,
2 DVE column-halo copies, and 2 small halo DMAs feeding the next step.
The banded stationary [3B+12, 5*3B] encodes all (ci, dy) taps; out
m = 3r + co matches the state layout so everything is lane-aligned.

Step 0 reads pre-swizzled fp16 records (host-prepared); the last step
writes f32 to HBM in (img, h, co, w) order, transposed back on host.
"""
import sys

sys.path.insert(0, "/opt/trn_rl_repo")

import numpy as np

N_CORES = 8
H = 1024
WC = 1024
WF = WC + 4
CG = 512
BLKS = [38] * 26 + [36]
NB = len(BLKS)
R0S = [0]
for _b in BLKS:
    R0S.append(R0S[-1] + _b)
assert R0S[-1] == H


def make_lhsT(W: np.ndarray, B: int) -> np.ndarray:
    """lhsT[p, 5*(3B)]: window partition p -> out m = 3r + co, per dx.

    Window layout (r-major, channel-interleaved): state row v in [0,B) at
    p = 3v + ci; next-halo rows v in {B,B+1} at p = 3B + 3(v-B) + ci;
    prev-halo rows v in {-2,-1} at p = 3B+6 + 3(v+2) + ci.
    """
    assert W.shape == (3, 3, 5, 5)
    KP = 3 * B + 12
    MP = 3 * B
    lhsT = np.zeros((KP, 5, MP), dtype=np.float32)
    for r in range(B):
        for dy in range(5):
            v = r + dy - 2
            for ci in range(3):
                if 0 <= v < B:
                    p = 3 * v + ci
                elif v >= B:
                    p = 3 * B + 3 * (v - B) + ci
                else:
                    p = 3 * B + 6 + 3 * (v + 2) + ci
                for dx in range(5):
                    for co in range(3):
                        lhsT[p, dx, 3 * r + co] = W[co, ci, dy, dx]
    return lhsT.reshape(KP, 5 * MP)


def prep_x(x: np.ndarray) -> np.ndarray:
    """(n,3,H,W) f32 -> (n, NB, 126, WF) bf16 block records in tile layout."""
    n = x.shape[0]
    xb = x.astype(np.float16)
    out = np.zeros((n, NB, 126, WF), dtype=np.float16)
    for b, B in enumerate(BLKS):
        r0 = R0S[b]
        rows = [(r0 + v) % H for v in range(B)]
        rows += [(r0 + B) % H, (r0 + B + 1) % H, (r0 - 2) % H, (r0 - 1) % H]
        # (n, 3, len(rows), W) -> r-major interleave (n, 3*len(rows), W)
        rec = xb[:, :, rows, :].transpose(0, 2, 1, 3).reshape(n, 3 * len(rows), WC)
        KP = 3 * B + 12
        out[:, b, :KP, 2 : WC + 2] = rec
        out[:, b, :KP, 0:2] = rec[:, :, WC - 2 : WC]
        out[:, b, :KP, WC + 2 : WC + 4] = rec[:, :, 0:2]
    return out


def build_body(tc, xprep_ap, lw38_ap, lw36_ap, y_ap, n_img, steps):
    from contextlib import ExitStack

    from concourse import mybir

    nc = tc.nc
    f32 = mybir.dt.float32
    f16 = mybir.dt.float16
    Relu = mybir.ActivationFunctionType.Relu

    ctx = ExitStack()
    const_pool = ctx.enter_context(tc.tile_pool(name="const", bufs=1))
    state_pool = ctx.enter_context(tc.tile_pool(name="state", bufs=1))
    t_pool = ctx.enter_context(tc.tile_pool(name="t", bufs=6))
    u_pool = ctx.enter_context(tc.tile_pool(name="u", bufs=6))
    y_pool = ctx.enter_context(tc.tile_pool(name="yst", bufs=6))
    psum_pool = ctx.enter_context(tc.tile_pool(name="psum", bufs=4, space="PSUM"))

    # persistent per-block state tiles (bufs=1 + unique names -> one slot
    # each), loaded once from the host-prepped records. The first two ride
    # the sync HWDGE queue ahead of everything else and the stationaries
    # load via the scalar queue, so block 0's matmuls start ~2us sooner;
    # the rest stream in on the SWDGE path.
    state = [
        [state_pool.tile([126, WF], f16, name=f"st{img}_{b}") for b in range(NB)]
        for img in range(n_img)
    ]
    nc.sync.dma_start(state[0][0][0:126, :], xprep_ap[0, 0, 0:126, :])
    nc.sync.dma_start(state[0][1][0:126, :], xprep_ap[0, 1, 0:126, :])

    lw38 = const_pool.tile([126, 5 * 114], f16)
    nc.scalar.dma_start(lw38[:], lw38_ap[:, :])
    lw36 = const_pool.tile([120, 5 * 108], f16)
    nc.scalar.dma_start(lw36[:], lw36_ap[:, :])

    for img in range(n_img):
        for b in range(NB):
            if img == 0 and b < 2:
                continue
            KP = 3 * BLKS[b] + 12
            nc.gpsimd.dma_start(state[img][b][0:KP, :], xprep_ap[img, b, 0:KP, :])

    for s in range(steps):
        last = s == steps - 1
        for img in range(n_img):
            for b in range(NB):
                B = BLKS[b]
                KP = 3 * B + 12
                MP = 3 * B
                R0 = R0S[b]
                lw = lw38 if B == 38 else lw36
                st = state[img][b]

                psums = [psum_pool.tile([114, CG], f32, name=f"ps{g}") for g in range(2)]
                for dx in range(5):
                    for g in range(2):
                        nc.tensor.matmul(
                            psums[g][0:MP, :],
                            lw[0:KP, MP * dx : MP * (dx + 1)],
                            st[0:KP, g * CG + dx : g * CG + dx + CG],
                            start=(dx == 0),
                            stop=(dx == 4),
                        )
                t = t_pool.tile([114, WC], f16)
                for g in range(2):
                    nc.scalar.activation(
                        t[0:MP, g * CG : (g + 1) * CG], psums[g][0:MP, :], Relu,
                        scale=0.1,
                    )
                u = u_pool.tile([114, WC], f16)
                nc.vector.tensor_add(u[0:MP, :], t[0:MP, :], st[0:MP, 2 : WC + 2])
                if last:
                    # fp16 output (host upcasts): keeps the min in the fast
                    # DVE mode and halves store bytes, so the trailing
                    # blocks' store chain doesn't back up after the last
                    # matmul; sync queue is idle in the last step
                    yt = y_pool.tile([114, WC], f16)
                    nc.vector.tensor_scalar_min(yt[0:MP, :], u[0:MP, :], 1.0)
                    nc.sync.dma_start(y_ap[img, R0 : R0 + B, :, :], yt[0:MP, :])
                    continue
                nc.vector.tensor_scalar_min(st[0:MP, 2 : WC + 2], u[0:MP, :], 1.0)
                nc.vector.tensor_copy(st[0:MP, 0:2], st[0:MP, WC : WC + 2])
                nc.vector.tensor_copy(st[0:MP, WC + 2 : WC + 4], st[0:MP, 2:4])
                # halo refreshes feeding step s+1 (single contiguous DMAs
                # thanks to the r-major interleave)
                if b >= 1:
                    Bp = BLKS[b - 1]
                    stp = state[img][b - 1]
                    nc.sync.dma_start(stp[3 * Bp : 3 * Bp + 6, :], st[0:6, :])
                    nc.sync.dma_start(
                        st[3 * B + 6 : 3 * B + 12, :], stp[3 * Bp - 6 : 3 * Bp, :]
                    )
                if b == NB - 1:
                    B0 = BLKS[0]
                    st0 = state[img][0]
                    nc.sync.dma_start(st[3 * B : 3 * B + 6, :], st0[0:6, :])
                    nc.sync.dma_start(
                        st0[3 * B0 + 6 : 3 * B0 + 12, :], st[3 * B - 6 : 3 * B, :]
                    )

    ctx.close()


_PROGRAM_CACHE = {}


def _build_program(n_img, steps):
    key = (n_img, steps)
    if key in _PROGRAM_CACHE:
        return _PROGRAM_CACHE[key]
    import concourse.tile as tile
    from concourse import bacc, mybir

    nc = bacc.Bacc(
        "TRN2",
        target_bir_lowering=False,
        debug=False,
        enable_asserts=False,
        num_devices=N_CORES,
    )
    f32 = mybir.dt.float32
    f16 = mybir.dt.float16
    xprep_ap = nc.dram_tensor(
        "xprep", (n_img, NB, 126, WF), f16, kind="ExternalInput"
    ).ap()
    lw38_ap = nc.dram_tensor("lw38", (126, 5 * 114), f16, kind="ExternalInput").ap()
    lw36_ap = nc.dram_tensor("lw36", (120, 5 * 108), f16, kind="ExternalInput").ap()
    y_ap = nc.dram_tensor("y", (n_img, H, 3, WC), f16, kind="ExternalOutput").ap()
    with tile.TileContext(nc) as tc:
        build_body(tc, xprep_ap, lw38_ap, lw36_ap, y_ap, n_img, steps)
    nc.compile()
    _PROGRAM_CACHE[key] = nc
    return nc


def kernel(x: np.ndarray, W: np.ndarray, steps) -> np.ndarray:
    from concourse.bass_utils import run_bass_kernel_spmd

    x = np.ascontiguousarray(np.asarray(x), dtype=np.float32)
    W = np.asarray(W, dtype=np.float32)
    steps = int(steps)
    n, c, Hx, Wx = x.shape
    assert c == 3 and Hx == H and Wx == WC and n % N_CORES == 0
    if steps == 0:
        return x
    per = n // N_CORES

    nc = _build_program(per, steps)
    xprep = prep_x(x)
    lw38 = make_lhsT(W, 38).astype(np.float16)
    lw36 = make_lhsT(W, 36).astype(np.float16)
    in_maps = [
        {"xprep": xprep[i * per : (i + 1) * per], "lw38": lw38, "lw36": lw36}
        for i in range(N_CORES)
    ]
    res = run_bass_kernel_spmd(nc, in_maps, core_ids=list(range(N_CORES)))
    y = np.concatenate([res.results[i]["y"] for i in range(N_CORES)], axis=0)
    return np.ascontiguousarray(y.transpose(0, 2, 1, 3)).astype(np.float32)


# revision 19
# speedup vs baseline: 1.0199x; 1.0008x over previous
"""Trainium2 Bass kernel for nn_CAKernel_47459388621075.

10 steps of x = clip(x + 0.1*relu(conv5x5_circular(x, W)), 0, 1) on
x:(16,3,1024,1024) f32, W:(3,3,5,5) f32.

Sharding: batch-parallel over 8 NeuronCores (2 images/core) — the circular
conv is per-image, so no cross-core communication is needed.

Per-core kernel: the whole state lives in SBUF as fp16 for all 10 steps
(12.6 MB; no HBM round-trip per step). Each image is split into 27 row
blocks (26x38 + 1x36 rows). A block's tile [3B+12, 1028] holds the state
rows r-major channel-interleaved (p = 3r + ci) plus 12 window-halo
partitions (rows B,B+1 then rows -2,-1, same interleave) and 2+2 circular
column halos, so every neighbor-halo refresh is a single contiguous
SBUF->SBUF DMA.

Per block per step: 10 fp16 matmuls (5 kernel-column taps x 2 PSUM groups
of 512 cols, dx-major so consecutive matmuls share the stationary matrix),
ACT relu+scale from PSUM, DVE add + clip-min writing the state in place,
2 DVE column-halo copies, and 2 small halo DMAs feeding the next step.
The banded stationary [3B+12, 5*3B] encodes all (ci, dy) taps; out
m = 3r + co matches the state layout so everything is lane-aligned.

Step 0 reads pre-swizzled fp16 records (host-prepared); the last step
writes f32 to HBM in (img, h, co, w) order, transposed back on host.
"""
import sys

sys.path.insert(0, "/opt/trn_rl_repo")

import numpy as np

N_CORES = 8
H = 1024
WC = 1024
WF = WC + 4
CG = 512
BLKS = [38] * 26 + [36]
NB = len(BLKS)
R0S = [0]
for _b in BLKS:
    R0S.append(R0S[-1] + _b)
assert R0S[-1] == H


def make_lhsT(W: np.ndarray, B: int) -> np.ndarray:
    """lhsT[p, 5*(3B)]: window partition p -> out m = 3r + co, per dx.

    Window layout (r-major, channel-interleaved): state row v in [0,B) at
    p = 3v + ci; next-halo rows v in {B,B+1} at p = 3B + 3(v-B) + ci;
    prev-halo rows v in {-2,-1} at p = 3B+6 + 3(v+2) + ci.
    """
    assert W.shape == (3, 3, 5, 5)
    KP = 3 * B + 12
    MP = 3 * B
    lhsT = np.zeros((KP, 5, MP), dtype=np.float32)
    for r in range(B):
        for dy in range(5):
            v = r + dy - 2
            for ci in range(3):
                if 0 <= v < B:
                    p = 3 * v + ci
                elif v >= B:
                    p = 3 * B + 3 * (v - B) + ci
                else:
                    p = 3 * B + 6 + 3 * (v + 2) + ci
                for dx in range(5):
                    for co in range(3):
                        lhsT[p, dx, 3 * r + co] = W[co, ci, dy, dx]
    return lhsT.reshape(KP, 5 * MP)


def prep_x(x: np.ndarray) -> np.ndarray:
    """(n,3,H,W) f32 -> (n, NB, 126, WF) bf16 block records in tile layout."""
    n = x.shape[0]
    xb = x.astype(np.float16)
    out = np.zeros((n, NB, 126, WF), dtype=np.float16)
    for b, B in enumerate(BLKS):
        r0 = R0S[b]
        rows = [(r0 + v) % H for v in range(B)]
        rows += [(r0 + B) % H, (r0 + B + 1) % H, (r0 - 2) % H, (r0 - 1) % H]
        # (n, 3, len(rows), W) -> r-major interleave (n, 3*len(rows), W)
        rec = xb[:, :, rows, :].transpose(0, 2, 1, 3).reshape(n, 3 * len(rows), WC)
        KP = 3 * B + 12
        out[:, b, :KP, 2 : WC + 2] = rec
        out[:, b, :KP, 0:2] = rec[:, :, WC - 2 : WC]
        out[:, b, :KP, WC + 2 : WC + 4] = rec[:, :, 0:2]
    return out


def build_body(tc, xprep_ap, lw38_ap, lw36_ap, y_ap, n_img, steps):
    from contextlib import ExitStack

    from concourse import mybir

    nc = tc.nc
    f32 = mybir.dt.float32
    f16 = mybir.dt.float16
    Relu = mybir.ActivationFunctionType.Relu

    ctx = ExitStack()
    const_pool = ctx.enter_context(tc.tile_pool(name="const", bufs=1))
    state_pool = ctx.enter_context(tc.tile_pool(name="state", bufs=1))
    t_pool = ctx.enter_context(tc.tile_pool(name="t", bufs=6))
    u_pool = ctx.enter_context(tc.tile_pool(name="u", bufs=6))
    y_pool = ctx.enter_context(tc.tile_pool(name="yst", bufs=6))
    psum_pool = ctx.enter_context(tc.tile_pool(name="psum", bufs=4, space="PSUM"))

    # persistent per-block state tiles (bufs=1 + unique names -> one slot
    # each), loaded once from the host-prepped records. The first two ride
    # the sync HWDGE queue ahead of everything else and the stationaries
    # load via the scalar queue, so block 0's matmuls start ~2us sooner;
    # the rest stream in on the SWDGE path.
    state = [
        [state_pool.tile([126, WF], f16, name=f"st{img}_{b}") for b in range(NB)]
        for img in range(n_img)
    ]
    nc.sync.dma_start(state[0][0][0:126, :], xprep_ap[0, 0, 0:126, :])
    nc.sync.dma_start(state[0][1][0:126, :], xprep_ap[0, 1, 0:126, :])

    lw38 = const_pool.tile([126, 5 * 114], f16)
    nc.scalar.dma_start(lw38[:], lw38_ap[:, :])
    lw36 = const_pool.tile([120, 5 * 108], f16)
    nc.scalar.dma_start(lw36[:], lw36_ap[:, :])

    for img in range(n_img):
        for b in range(NB):
            if img == 0 and b < 2:
                continue
            KP = 3 * BLKS[b] + 12
            nc.gpsimd.dma_start(state[img][b][0:KP, :], xprep_ap[img, b, 0:KP, :])

    for s in range(steps):
        last = s == steps - 1
        for img in range(n_img):
            for b in range(NB):
                B = BLKS[b]
                KP = 3 * B + 12
                MP = 3 * B
                R0 = R0S[b]
                lw = lw38 if B == 38 else lw36
                st = state[img][b]

                psums = [psum_pool.tile([114, CG], f32, name=f"ps{g}") for g in range(2)]
                for dx in range(5):
                    for g in range(2):
                        nc.tensor.matmul(
                            psums[g][0:MP, :],
                            lw[0:KP, MP * dx : MP * (dx + 1)],
                            st[0:KP, g * CG + dx : g * CG + dx + CG],
                            start=(dx == 0),
                            stop=(dx == 4),
                        )
                t = t_pool.tile([114, WC], f16)
                for g in range(2):
                    nc.scalar.activation(
                        t[0:MP, g * CG : (g + 1) * CG], psums[g][0:MP, :], Relu,
                        scale=0.1,
                    )
                u = u_pool.tile([114, WC], f16)
                nc.vector.tensor_add(u[0:MP, :], t[0:MP, :], st[0:MP, 2 : WC + 2])
                if last:
                    # fp16 output (host upcasts): keeps the min in the fast
                    # DVE mode and halves store bytes, so the trailing
                    # blocks' store chain doesn't back up after the last
                    # matmul; sync queue is idle in the last step
                    yt = y_pool.tile([114, WC], f16)
                    nc.vector.tensor_scalar_min(yt[0:MP, :], u[0:MP, :], 1.0)
                    nc.sync.dma_start(y_ap[img, R0 : R0 + B, :, :], yt[0:MP, :])
                    continue
                nc.vector.tensor_scalar_min(st[0:MP, 2 : WC + 2], u[0:MP, :], 1.0)
                nc.vector.tensor_copy(st[0:MP, 0:2], st[0:MP, WC : WC + 2])
                nc.vector.tensor_copy(st[0:MP, WC + 2 : WC + 4], st[0:MP, 2:4])
                # halo refreshes feeding step s+1 (single contiguous DMAs
                # thanks to the r-major interleave)
                if b >= 1:
                    Bp = BLKS[b - 1]
                    stp = state[img][b - 1]
                    nc.sync.dma_start(stp[3 * Bp : 3 * Bp + 6, :], st[0:6, :])
                    nc.sync.dma_start(
                        st[3 * B + 6 : 3 * B + 12, :], stp[3 * Bp - 6 : 3 * Bp, :]
                    )
                if b == NB - 1:
                    B0 = BLKS[0]
                    st0 = state[img][0]
                    nc.sync.dma_start(st[3 * B : 3 * B + 6, :], st0[0:6, :])
                    nc.sync.dma_start(
                        st0[3 * B0 + 6 : 3 * B0 + 12, :], st[3 * B - 6 : 3 * B, :]
                    )

    ctx.close()


_PROGRAM_CACHE = {}


def _build_program(n_img, steps):
    key = (n_img, steps)
    if key in _PROGRAM_CACHE:
        return _PROGRAM_CACHE[key]
    import concourse.tile as tile
    from concourse import bacc, mybir

    nc = bacc.Bacc(
        "TRN2",
        target_bir_lowering=False,
        debug=False,
        enable_asserts=False,
        num_devices=N_CORES,
    )
    f32 = mybir.dt.float32
    f16 = mybir.dt.float16
    xprep_ap = nc.dram_tensor(
        "xprep", (n_img, NB, 126, WF), f16, kind="ExternalInput"
    ).ap()
    lw38_ap = nc.dram_tensor("lw38", (126, 5 * 114), f16, kind="ExternalInput").ap()
    lw36_ap = nc.dram_tensor("lw36", (120, 5 * 108), f16, kind="ExternalInput").ap()
    y_ap = nc.dram_tensor("y", (n_img, H, 3, WC), f16, kind="ExternalOutput").ap()
    with tile.TileContext(nc) as tc:
        build_body(tc, xprep_ap, lw38_ap, lw36_ap, y_ap, n_img, steps)
    nc.compile()
    _PROGRAM_CACHE[key] = nc
    return nc


def kernel(x: np.ndarray, W: np.ndarray, steps) -> np.ndarray:
    from concourse.bass_utils import run_bass_kernel_spmd

    x = np.ascontiguousarray(np.asarray(x), dtype=np.float32)
    W = np.asarray(W, dtype=np.float32)
    steps = int(steps)
    n, c, Hx, Wx = x.shape
    assert c == 3 and Hx == H and Wx == WC and n % N_CORES == 0
    if steps == 0:
        return x
    per = n // N_CORES

    nc = _build_program(per, steps)
    xprep = prep_x(x)
    lw38 = make_lhsT(W, 38).astype(np.float16)
    lw36 = make_lhsT(W, 36).astype(np.float16)
    in_maps = [
        {"xprep": xprep[i * per : (i + 1) * per], "lw38": lw38, "lw36": lw36}
        for i in range(N_CORES)
    ]
    res = run_bass_kernel_spmd(nc, in_maps, core_ids=list(range(N_CORES)))
    y = np.concatenate([res.results[i]["y"] for i in range(N_CORES)], axis=0)
    return np.ascontiguousarray(y.transpose(0, 2, 1, 3)).astype(np.float32)
